# revision 21
# baseline (speedup 1.0000x reference)
"""Trainium2 Bass kernel for nn_EncoderLayer_42399917146737.

The reference "SSM scan" is degenerate: at every step i the recurrence
overwrites h at exactly the positions p with pc[p,i]==1 with the scalar
b_i, and the step output reads only those positions.  Hence

    y_i[b] = C[b,i] * Bcoef[b,i] * n_i,      n_i = sum_p pc[p,i]

with no sequential dependence, and the reverse scan equals the forward
one.  The broadcast over p then reduces the Wr projection to a scalar
sum, so the whole module collapses to

    logits[b,l] = 2*sum(Wr) * has_err[b] * n_l * C[b,l] * (Bbias[b,l]/M + tanh(|X[b,l]|*wb_l))
    out         = softmax_l(logits)

where  Bbias = h0 @ pc,  h0 = 1-2*parity(hard @ pc^T),  hard = (X<0),
M = max|Bbias| (GLOBAL over the full batch),  wb = Wb @ pc,  wc = Wc @ pc,
C = 0.5 + tanh(|X|*wc_l).  (br shifts all logits equally -> drops out of
softmax.)

Sharding: batch B=128 over 8 cores (16 rows each).  Because M is a
global max over the whole batch, every core recomputes the (cheap)
full-batch parity/Bbias matmuls; the per-batch elementwise work + softmax
run only on the core's own 16 rows.  Per-core batch selection is done
with a per-core one-hot selection matrix (E_c) fed through the tensor
engine, so a single NEFF serves all 8 cores.

Precision: pc/hard/m are {0,1} so fp8/bf16 matmuls with f32 accumulate
are exact; X^T for sign tests rides in bf16 (sign-exact); Wb/Wc ride the
bf16 `pcl` matmul as hi+lo split columns (~2^-16 rel err); the local
elementwise path keeps full f32 X.
"""

import numpy as np
import ml_dtypes

B, L, P = 128, 1024, 512
NCORES = 8
BS = B // NCORES  # 16
LT = L // 128     # 8 L-tiles
PT = P // 128     # 4 P-tiles

_cache = {}


def _build_nc():
    import concourse.bass as bass
    import concourse.bacc as bacc
    import concourse.tile as tile
    from concourse import mybir

    f32 = mybir.dt.float32
    bf16 = mybir.dt.bfloat16
    fp8 = mybir.dt.float8e4
    u32 = mybir.dt.uint32
    Alu = mybir.AluOpType
    Act = mybir.ActivationFunctionType
    Ax = mybir.AxisListType

    nc = bacc.Bacc("TRN2", target_bir_lowering=False, debug=False)

    # ---- DRAM I/O (host pre-swizzles everything partition-major) ----
    xtb_d = nc.dram_tensor("xtb", (128, L), bf16, kind="ExternalInput")
    pct_d = nc.dram_tensor("pct", (128, LT * P), fp8, kind="ExternalInput")
    pcl_d = nc.dram_tensor("pcl", (128, PT * L), fp8, kind="ExternalInput")
    # bigf: [xl 0:128 | ec 128:144 | wt 144:152 | wr 152:156 | idn 156:284]
    NF = 284
    big_d = nc.dram_tensor("big", (128, NF), f32, kind="ExternalInput")
    y_d = nc.dram_tensor("y", (BS, L), f32, kind="ExternalOutput")

    NW = 9                    # wb0 wc0 wb1 wc1 wb2 wc2 wb3 wc3 | ones
    NB = 128 + BS             # m^T | m^T_loc
    NR = NB + NW              # combined-matmul rhs width
    HLT = LT // 2

    def bcast(col_ap, n):
        """Free-dim step-0 broadcast of a (...,1) AP to (...,n)."""
        return bass.AP(tensor=col_ap.tensor, offset=col_ap.offset,
                       ap=[*col_ap.ap[:-1], [0, n]])

    with tile.TileContext(nc) as tc:
        with (
            tc.tile_pool(name="sb", bufs=1) as sb,
            tc.tile_pool(name="ps", bufs=3, space="PSUM") as ps,
            tc.tile_pool(name="ps2", bufs=2, space="PSUM") as ps2,
            tc.tile_pool(name="ps4", bufs=1, space="PSUM") as ps4,
            tc.tile_pool(name="ps3", bufs=1, space="PSUM") as ps3,
        ):
            XTB = sb.tile([128, LT, 128], bf16)
            PCT = sb.tile([128, LT, P], fp8)
            PCL = sb.tile([128, PT, L], fp8)
            BIG = sb.tile([128, NF], f32)
            XL = BIG[:, 0:128].rearrange("p (i j) -> p i j", i=LT)
            EC = BIG[:, 128:144]
            WT = BIG[:, 144:152].rearrange("p (k t) -> p k t", k=PT)
            WRp = BIG[:, 152:156]
            IDN = BIG[:, 156:284]
            # One HWDGE ring; FIFO order = transfer priority.
            nc.sync.dma_start(XTB[:].rearrange("p i b -> p (i b)"), xtb_d[:])
            nc.sync.dma_start(PCT[:, 0:4, :].rearrange("p i q -> p (i q)"),
                              pct_d[:, 0:4 * P])
            nc.sync.dma_start(PCT[:, 4:8, :].rearrange("p i q -> p (i q)"),
                              pct_d[:, 4 * P:8 * P])
            nc.sync.dma_start(BIG[:], big_d[:])
            nc.sync.dma_start(PCL[:].rearrange("p k l -> p (k l)"), pcl_d[:])

            # ---- hard decisions (transposed, fp8 {0,1}) ----
            HT = sb.tile([128, LT, 128], fp8)
            nc.vector.tensor_scalar(
                HT[:].rearrange("p i b -> p (i b)"),
                XTB[:].rearrange("p i b -> p (i b)"),
                0.0, None, Alu.is_lt)

            # ---- syndrome counts: S[b,q] = sum_l hard[b,l]*pc[q,l] ----
            S_ps = ps.tile([128, P], f32, tag="mm")
            for g in range(LT // 2):
                nc.tensor.matmul(S_ps[:], HT[:, 2 * g:2 * g + 2, :],
                                 PCT[:, 2 * g:2 * g + 2, :],
                                 perf_mode=mybir.MatmulPerfMode.DoubleRow,
                                 start=(g == 0), stop=(g == LT // 2 - 1))

            # ---- combined rhs (fp8): [ m^T | m^T_loc | W 4-term splits | ones ]
            # Wb/Wc are carried as 4 scaled fp8 terms each: w = sum_k t_k/16^k,
            # with t_k stored as fp8(residual_k * 16^k) so terms stay in
            # fp8's normal range.  Reconstruction happens after the matmul.
            RHS = sb.tile([128, PT, NR], fp8)
            R1 = sb.tile([128, PT, 2], f32)
            R2 = sb.tile([128, PT, 2], f32)
            R3 = sb.tile([128, PT, 2], f32)
            for k in range(PT):
                nc.scalar.copy(RHS[:, k, NB:NB + 2], WT[:, k, :])            # t0
                nc.vector.tensor_tensor(R1[:, k, :], WT[:, k, :],
                                        RHS[:, k, NB:NB + 2], Alu.subtract)
                nc.vector.tensor_scalar(RHS[:, k, NB + 2:NB + 4], R1[:, k, :],
                                        16.0, None, Alu.mult)                # t1
                nc.vector.scalar_tensor_tensor(R2[:, k, :],
                                               RHS[:, k, NB + 2:NB + 4],
                                               -1.0 / 16.0, R1[:, k, :],
                                               Alu.mult, Alu.add)
                nc.vector.tensor_scalar(RHS[:, k, NB + 4:NB + 6], R2[:, k, :],
                                        256.0, None, Alu.mult)               # t2
                nc.vector.scalar_tensor_tensor(R3[:, k, :],
                                               RHS[:, k, NB + 4:NB + 6],
                                               -1.0 / 256.0, R2[:, k, :],
                                               Alu.mult, Alu.add)
                nc.vector.tensor_scalar(RHS[:, k, NB + 6:NB + 8], R3[:, k, :],
                                        4096.0, None, Alu.mult)              # t3
                nc.vector.memset(RHS[:, k, NB + 8:NB + 9], 1.0)              # ones
            # early scalar chain: 2*sum(Wr) broadcast (independent of parity/M)
            ONES1 = sb.tile([1, 128], f32)
            nc.vector.memset(ONES1[:], 1.0)
            ONESC = sb.tile([128, 1], f32)
            nc.vector.memset(ONESC[:], 1.0)
            wrs = sb.tile([128, 1], f32)
            nc.vector.reduce_sum(wrs[:], WRp, axis=Ax.X)
            swr_ps = ps4.tile([1, 1], f32, tag="tp2")
            nc.tensor.matmul(swr_ps[:], wrs[:], ONESC[:])
            SWR = sb.tile([1, 1], f32)
            nc.vector.tensor_scalar(SWR[:], swr_ps[:], 2.0, None, Alu.mult)
            sb2_ps = ps4.tile([128, 1], f32, tag="tp2")
            nc.tensor.matmul(sb2_ps[:], ONES1[:], SWR[:])
            SCs2 = sb.tile([128, 1], f32)
            nc.scalar.copy(SCs2[:], sb2_ps[:])

            # parity m = S mod 2 (exact integer bit trick), chunked for overlap
            mag = sb.tile([128, P], f32)
            magu = sb.tile([128, P], u32)
            m_f = sb.tile([128, P], f32)
            for k in range(PT):
                ck = slice(k * 128, (k + 1) * 128)
                nc.vector.tensor_scalar(mag[:, ck], S_ps[:, ck], float(2 ** 23),
                                        None, Alu.add)
                nc.vector.tensor_scalar(magu[:, ck], mag[:, ck].bitcast(u32), 1,
                                        None, Alu.bitwise_and)
                nc.vector.tensor_copy(m_f[:, ck], magu[:, ck])
                mt_ps = ps2.tile([128, 128], f32, tag="tp")
                nc.tensor.transpose(mt_ps[:], m_f[:, ck], IDN)
                nc.scalar.copy(RHS[:, k, 0:128], mt_ps[:])
                ml_ps = ps4.tile([128, BS], f32, tag="tp2")
                nc.tensor.matmul(ml_ps[:], m_f[:, ck], EC)
                nc.scalar.copy(RHS[:, k, 128:NB], ml_ps[:])
            cnt = sb.tile([128, 1], f32)
            nc.vector.reduce_sum(cnt[:], m_f[:], axis=Ax.X)
            # per-row scale: alpha = 2*sum(Wr)*has_err (local rows; early)
            cl_ps = ps4.tile([BS, 1], f32, tag="tp2")
            nc.tensor.matmul(cl_ps[:], EC, cnt[:])
            HE = sb.tile([BS, 1], f32)
            nc.vector.tensor_scalar(HE[:], cl_ps[:], 0.0, None, Alu.is_gt)
            AL = sb.tile([BS, 1], f32)
            nc.vector.tensor_tensor(AL[:], HE[:], SCs2[0:BS, 0:1], Alu.mult)

            # ---- combined matmul over P (fp8 DoubleRow):  OUT = pc^T @ RHS ----
            WBA = sb.tile([128, LT, NW], f32)    # raw W-term columns + n
            WBCN = sb.tile([128, LT, 2], f32)    # reconstructed wb, wc per l
            BBT = sb.tile([128, LT, NB], f32)    # Bbias^T: full batch | local
            AMX = sb.tile([128, LT], f32)
            for t in range(LT):
                out_ps = ps.tile([128, NR], f32, tag="mm")
                for g in range(PT // 2):
                    nc.tensor.matmul(out_ps[:],
                                     PCL[:, 2 * g:2 * g + 2, t * 128:(t + 1) * 128],
                                     RHS[:, 2 * g:2 * g + 2, :],
                                     perf_mode=mybir.MatmulPerfMode.DoubleRow,
                                     start=(g == 0), stop=(g == PT // 2 - 1))
                nc.scalar.copy(WBA[:, t, :], out_ps[:, NB:NB + NW])
                # wb,wc = ((t3/16 + t2)/16 + t1)/16 + t0   (gpsimd: off DVE)
                nc.vector.scalar_tensor_tensor(WBCN[:, t, :], WBA[:, t, 6:8],
                                               1.0 / 16.0, WBA[:, t, 4:6],
                                               Alu.mult, Alu.add)
                nc.vector.scalar_tensor_tensor(WBCN[:, t, :], WBCN[:, t, :],
                                               1.0 / 16.0, WBA[:, t, 2:4],
                                               Alu.mult, Alu.add)
                nc.vector.scalar_tensor_tensor(WBCN[:, t, :], WBCN[:, t, :],
                                               1.0 / 16.0, WBA[:, t, 0:2],
                                               Alu.mult, Alu.add)
                nc.vector.tensor_scalar(BBT[:, t, :], out_ps[:, 0:NB],
                                        -2.0, WBA[:, t, 8:9], Alu.mult, Alu.add)
                nc.vector.tensor_reduce(AMX[:, t:t + 1], BBT[:, t, 0:128], axis=Ax.X,
                                        op=Alu.max, apply_absolute_value=True)

            # ---- global 1/M, broadcast to partitions ----
            AMXr = sb.tile([128, 1], f32)
            nc.vector.tensor_reduce(AMXr[:], AMX[:], axis=Ax.X, op=Alu.max)
            tr_ps = ps4.tile([1, 128], f32, tag="tp2")
            nc.tensor.transpose(tr_ps[:], AMXr[:], IDN)
            Mg = sb.tile([1, 1], f32)
            nc.vector.tensor_reduce(Mg[:], tr_ps[:], axis=Ax.X, op=Alu.max)
            sb1_ps = ps4.tile([128, 1], f32, tag="tp2")
            nc.tensor.matmul(sb1_ps[:], ONES1[:], Mg[:])
            SCs1 = sb.tile([128, 1], f32)
            nc.vector.reciprocal(SCs1[:], sb1_ps[:])

            # ---- local elementwise (6+2 split: big part overlaps combined) ----
            XA = sb.tile([128, LT, BS], f32)
            A1 = sb.tile([128, LT, BS], f32)
            T1 = sb.tile([128, LT, BS], f32)
            A2 = sb.tile([128, LT, BS], f32)
            C2 = sb.tile([128, LT, BS], f32)
            U = sb.tile([128, LT, BS], f32)
            W = sb.tile([128, LT, BS], f32)
            Q8 = sb.tile([128, LT, BS], f32)
            qt_psA = ps3.tile([BS, 512], f32, tag="qta")
            qt_psB = ps3.tile([BS, 512], f32, tag="qtb")

            def flat(ap):
                return ap.rearrange("p i j -> p (i j)")

            # Everything except the 1/M scaling is M-independent:
            #   logits = alpha * (G1 + (1/M)*G2),  G1 = n*C*t1,  G2 = n*C*Bbias
            # G1 transposes start during the combined phase; the 1/M*G2 term
            # is folded in afterwards by accumulating G2 @ diag(1/M) onto the
            # same PSUM banks.
            CN = U   # reuse tiles
            G1 = W
            G2 = Q8
            for s0 in range(0, LT, 2):
                hs = slice(s0, s0 + 2)
                nc.scalar.activation(flat(XA[:, hs, :]), flat(XL[:, hs, :]), Act.Abs)
                nc.vector.tensor_tensor(A1[:, hs, :], XA[:, hs, :],
                                        bcast(WBCN[:, hs, 0:1], BS), Alu.mult)
                nc.scalar.activation(flat(T1[:, hs, :]), flat(A1[:, hs, :]), Act.Tanh)
                nc.vector.tensor_tensor(A2[:, hs, :], XA[:, hs, :],
                                        bcast(WBCN[:, hs, 1:2], BS), Alu.mult)
                nc.scalar.activation(flat(C2[:, hs, :]), flat(A2[:, hs, :]), Act.Tanh)
                nc.vector.scalar_tensor_tensor(CN[:, hs, :], C2[:, hs, :], 0.5,
                                               bcast(WBA[:, hs, 8:9], BS),
                                               Alu.add, Alu.mult)
                nc.vector.tensor_tensor(G1[:, hs, :], CN[:, hs, :], T1[:, hs, :],
                                        Alu.mult)
                nc.vector.tensor_tensor(G2[:, hs, :], CN[:, hs, :],
                                        BBT[:, hs, 128:NB], Alu.mult)
            # post-M: QS = G1 + (1/M)*G2, then transpose per tile
            QS = sb.tile([128, LT, BS], f32)
            nc.vector.scalar_tensor_tensor(QS[:], G2[:], SCs1[:, 0:1], G1[:],
                                           Alu.mult, Alu.add)
            for j in range(LT):
                qp, jo = (qt_psA, j) if j < 4 else (qt_psB, j - 4)
                nc.tensor.transpose(qp[:, jo * 128:(jo + 1) * 128], QS[:, j, :], IDN)

            # ---- alpha & softmax (halves split across ACT and DVE) ----
            QFa = sb.tile([BS, 512], f32)
            QFb = sb.tile([BS, 512], f32)
            nc.scalar.activation(QFa[:], qt_psA[:], Act.Copy, scale=AL[:, 0:1])
            nc.vector.tensor_scalar(QFb[:], qt_psB[:], AL[:, 0:1], None, Alu.mult)
            nma = sb.tile([BS, 1], f32)
            nmb = sb.tile([BS, 1], f32)
            nc.vector.tensor_reduce(nmb[:], QFb[:], axis=Ax.X, op=Alu.max, negate=True)
            nc.vector.tensor_reduce(nma[:], QFa[:], axis=Ax.X, op=Alu.max, negate=True)
            nmx = sb.tile([BS, 1], f32)
            nc.vector.tensor_tensor(nmx[:], nma[:], nmb[:], Alu.min)
            EXa = sb.tile([BS, 512], f32)
            EXb = sb.tile([BS, 512], f32)
            ssa = sb.tile([BS, 1], f32)
            ssb = sb.tile([BS, 1], f32)
            nc.scalar.activation(EXb[:], QFb[:], Act.Exp, bias=nmx[:, 0:1], scale=1.0)
            nc.vector.reduce_sum(ssb[:], EXb[:], axis=Ax.X)
            nc.scalar.activation(EXa[:], QFa[:], Act.Exp, bias=nmx[:, 0:1], scale=1.0)
            nc.vector.reduce_sum(ssa[:], EXa[:], axis=Ax.X)
            ssum = sb.tile([BS, 1], f32)
            nc.vector.tensor_tensor(ssum[:], ssa[:], ssb[:], Alu.add)
            rs = sb.tile([BS, 1], f32)
            nc.vector.reciprocal(rs[:], ssum[:])
            OUTa = sb.tile([BS, 512], f32)
            OUTb = sb.tile([BS, 512], f32)
            nc.vector.tensor_scalar(OUTb[:], EXb[:], rs[:, 0:1], None, Alu.mult)
            nc.scalar.activation(OUTa[:], EXa[:], Act.Copy, scale=rs[:, 0:1])
            nc.sync.dma_start(y_d[:, 0:512], OUTa[:])
            nc.sync.dma_start(y_d[:, 512:1024], OUTb[:])

    nc.compile()
    return nc


def _prep_in_maps(X, pc_matrix, Wb, Wc, Wr, br):
    bf16 = ml_dtypes.bfloat16
    fp8 = ml_dtypes.float8_e4m3
    X = np.ascontiguousarray(np.asarray(X, dtype=np.float32))
    pc = np.asarray(pc_matrix)
    xT = X[:, :, 0].T  # (L, B)

    xtb = np.ascontiguousarray(
        xT.astype(bf16).reshape(LT, 128, B).transpose(1, 0, 2).reshape(128, LT * B))
    pct = np.ascontiguousarray(
        pc.T.astype(fp8).reshape(LT, 128, P).transpose(1, 0, 2).reshape(128, LT * P))
    pcl = np.ascontiguousarray(
        pc.astype(fp8).reshape(PT, 128, L).transpose(1, 0, 2).reshape(128, PT * L))
    w3 = np.stack([np.asarray(Wb, dtype=np.float32)[0],
                   np.asarray(Wc, dtype=np.float32)[0]], axis=1)  # (P, 2)
    wt = w3.reshape(PT, 128, 2).transpose(1, 0, 2).reshape(128, PT * 2)
    wrp = np.asarray(Wr, dtype=np.float32).reshape(128, 4)
    idn = np.eye(128, dtype=np.float32)

    in_maps = []
    for c in range(NCORES):
        sel = slice(c * BS, (c + 1) * BS)
        ec = np.zeros((128, BS), dtype=np.float32)
        ec[np.arange(c * BS, (c + 1) * BS), np.arange(BS)] = 1.0
        xl = xT[:, sel].reshape(LT, 128, BS).transpose(1, 0, 2).reshape(128, LT * BS)
        big = np.concatenate([xl, ec, wt, wrp, idn], axis=1).astype(np.float32)
        assert big.shape == (128, 284)
        in_maps.append({"xtb": xtb, "pct": pct, "pcl": pcl,
                        "big": np.ascontiguousarray(big)})
    return in_maps


def run(inputs, trace=False, **kw):
    if "nc" not in _cache:
        _cache["nc"] = _build_nc()
    nc = _cache["nc"]
    in_maps = _prep_in_maps(**inputs)
    from concourse.bass_utils import run_bass_kernel_spmd
    res = run_bass_kernel_spmd(nc, in_maps, core_ids=list(range(NCORES)),
                               trace=trace, **kw)
    out = np.concatenate([res.results[c]["y"] for c in range(NCORES)], axis=0)
    return np.ascontiguousarray(out[:, :, None].astype(np.float32)), res


def kernel(**inputs) -> np.ndarray:
    out, _ = run(inputs)
    return out


# revision 22
# speedup vs baseline: 1.0468x; 1.0468x over previous
"""Trainium2 Bass kernel for nn_EncoderLayer_42399917146737.

The reference "SSM scan" is degenerate: at every step i the recurrence
overwrites h at exactly the positions p with pc[p,i]==1 with the scalar
b_i, and the step output reads only those positions.  Hence

    y_i[b] = C[b,i] * Bcoef[b,i] * n_i,      n_i = sum_p pc[p,i]

with no sequential dependence, and the reverse scan equals the forward
one.  The broadcast over p then reduces the Wr projection to a scalar
sum, so the whole module collapses to

    logits[b,l] = 2*sum(Wr) * has_err[b] * n_l * C[b,l] * (Bbias[b,l]/M + tanh(|X[b,l]|*wb_l))
    out         = softmax_l(logits)

where  Bbias = h0 @ pc,  h0 = 1-2*parity(hard @ pc^T),  hard = (X<0),
M = max|Bbias| (GLOBAL over the full batch),  wb = Wb @ pc,  wc = Wc @ pc,
C = 0.5 + tanh(|X|*wc_l).  (br shifts all logits equally -> drops out of
softmax.)

Sharding: batch B=128 over 8 cores (16 rows each).  Because M is a
global max over the whole batch, every core recomputes the (cheap)
full-batch parity/Bbias matmuls; the per-batch elementwise work + softmax
run only on the core's own 16 rows.  Per-core batch selection is done
with a per-core one-hot selection matrix (E_c) fed through the tensor
engine, so a single NEFF serves all 8 cores.

Precision: pc/hard/m are {0,1} so fp8/bf16 matmuls with f32 accumulate
are exact; X^T for sign tests rides in bf16 (sign-exact); Wb/Wc ride the
bf16 `pcl` matmul as hi+lo split columns (~2^-16 rel err); the local
elementwise path keeps full f32 X.
"""

import numpy as np
import ml_dtypes

B, L, P = 128, 1024, 512
NCORES = 8
BS = B // NCORES  # 16
LT = L // 128     # 8 L-tiles
PT = P // 128     # 4 P-tiles

_cache = {}


def _build_nc():
    import concourse.bass as bass
    import concourse.bacc as bacc
    import concourse.tile as tile
    from concourse import mybir

    f32 = mybir.dt.float32
    bf16 = mybir.dt.bfloat16
    fp8 = mybir.dt.float8e4
    u32 = mybir.dt.uint32
    Alu = mybir.AluOpType
    Act = mybir.ActivationFunctionType
    Ax = mybir.AxisListType

    nc = bacc.Bacc("TRN2", target_bir_lowering=False, debug=False)

    # ---- DRAM I/O (host pre-swizzles everything partition-major) ----
    xtb_d = nc.dram_tensor("xtb", (128, L), bf16, kind="ExternalInput")
    pct_d = nc.dram_tensor("pct", (128, LT * P), fp8, kind="ExternalInput")
    pcl_d = nc.dram_tensor("pcl", (128, PT * L), fp8, kind="ExternalInput")
    # bigf: [xl 0:128 | ec 128:144 | wt 144:152 | wr 152:156 | idn 156:284]
    NF = 284
    big_d = nc.dram_tensor("big", (128, NF), f32, kind="ExternalInput")
    y_d = nc.dram_tensor("y", (BS, L), f32, kind="ExternalOutput")

    NW = 9                    # wb0 wc0 wb1 wc1 wb2 wc2 wb3 wc3 | ones
    NB = 128 + BS             # m^T | m^T_loc
    NR = NB + NW              # combined-matmul rhs width
    HLT = LT // 2

    def bcast(col_ap, n):
        """Free-dim step-0 broadcast of a (...,1) AP to (...,n)."""
        return bass.AP(tensor=col_ap.tensor, offset=col_ap.offset,
                       ap=[*col_ap.ap[:-1], [0, n]])

    with tile.TileContext(nc) as tc:
        with (
            tc.tile_pool(name="sb", bufs=1) as sb,
            tc.tile_pool(name="ps", bufs=3, space="PSUM") as ps,
            tc.tile_pool(name="ps2", bufs=2, space="PSUM") as ps2,
            tc.tile_pool(name="ps4", bufs=1, space="PSUM") as ps4,
            tc.tile_pool(name="ps3", bufs=1, space="PSUM") as ps3,
        ):
            XTB = sb.tile([128, LT, 128], bf16)
            PCT = sb.tile([128, LT, P], fp8)
            PCL = sb.tile([128, PT, L], fp8)
            BIG = sb.tile([128, NF], f32)
            XL = BIG[:, 0:128].rearrange("p (i j) -> p i j", i=LT)
            EC = BIG[:, 128:144]
            WT = BIG[:, 144:152].rearrange("p (k t) -> p k t", k=PT)
            WRp = BIG[:, 152:156]
            IDN = BIG[:, 156:284]
            # One HWDGE ring; FIFO order = transfer priority.
            nc.sync.dma_start(XTB[:, 0:4, :].rearrange("p i b -> p (i b)"),
                              xtb_d[:, 0:512])
            nc.sync.dma_start(PCT[:, 0:4, :].rearrange("p i q -> p (i q)"),
                              pct_d[:, 0:4 * P])
            nc.sync.dma_start(XTB[:, 4:8, :].rearrange("p i b -> p (i b)"),
                              xtb_d[:, 512:1024])
            nc.sync.dma_start(PCT[:, 4:8, :].rearrange("p i q -> p (i q)"),
                              pct_d[:, 4 * P:8 * P])
            nc.sync.dma_start(BIG[:], big_d[:])
            nc.sync.dma_start(PCL[:].rearrange("p k l -> p (k l)"), pcl_d[:])

            # ---- hard decisions (transposed, fp8 {0,1}) ----
            HT = sb.tile([128, LT, 128], fp8)
            for h in range(2):
                nc.vector.tensor_scalar(
                    HT[:, 4 * h:4 * h + 4, :].rearrange("p i b -> p (i b)"),
                    XTB[:, 4 * h:4 * h + 4, :].rearrange("p i b -> p (i b)"),
                    0.0, None, Alu.is_lt)

            # ---- syndrome counts: S[b,q] = sum_l hard[b,l]*pc[q,l] ----
            S_ps = ps.tile([128, P], f32, tag="mm")
            for g in range(LT // 2):
                nc.tensor.matmul(S_ps[:], HT[:, 2 * g:2 * g + 2, :],
                                 PCT[:, 2 * g:2 * g + 2, :],
                                 perf_mode=mybir.MatmulPerfMode.DoubleRow,
                                 start=(g == 0), stop=(g == LT // 2 - 1))

            # ---- combined rhs (fp8): [ m^T | m^T_loc | W 4-term splits | ones ]
            # Wb/Wc are carried as 4 scaled fp8 terms each: w = sum_k t_k/16^k,
            # with t_k stored as fp8(residual_k * 16^k) so terms stay in
            # fp8's normal range.  Reconstruction happens after the matmul.
            RHS = sb.tile([128, PT, NR], fp8)
            R1 = sb.tile([128, PT, 2], f32)
            R2 = sb.tile([128, PT, 2], f32)
            R3 = sb.tile([128, PT, 2], f32)
            for k in range(PT):
                nc.scalar.copy(RHS[:, k, NB:NB + 2], WT[:, k, :])            # t0
                nc.vector.tensor_tensor(R1[:, k, :], WT[:, k, :],
                                        RHS[:, k, NB:NB + 2], Alu.subtract)
                nc.vector.tensor_scalar(RHS[:, k, NB + 2:NB + 4], R1[:, k, :],
                                        16.0, None, Alu.mult)                # t1
                nc.vector.scalar_tensor_tensor(R2[:, k, :],
                                               RHS[:, k, NB + 2:NB + 4],
                                               -1.0 / 16.0, R1[:, k, :],
                                               Alu.mult, Alu.add)
                nc.vector.tensor_scalar(RHS[:, k, NB + 4:NB + 6], R2[:, k, :],
                                        256.0, None, Alu.mult)               # t2
                nc.vector.scalar_tensor_tensor(R3[:, k, :],
                                               RHS[:, k, NB + 4:NB + 6],
                                               -1.0 / 256.0, R2[:, k, :],
                                               Alu.mult, Alu.add)
                nc.vector.tensor_scalar(RHS[:, k, NB + 6:NB + 8], R3[:, k, :],
                                        4096.0, None, Alu.mult)              # t3
                nc.vector.memset(RHS[:, k, NB + 8:NB + 9], 1.0)              # ones
            # early scalar chain: 2*sum(Wr) broadcast (independent of parity/M)
            ONES1 = sb.tile([1, 128], f32)
            nc.vector.memset(ONES1[:], 1.0)
            ONESC = sb.tile([128, 1], f32)
            nc.vector.memset(ONESC[:], 1.0)
            wrs = sb.tile([128, 1], f32)
            nc.vector.reduce_sum(wrs[:], WRp, axis=Ax.X)
            swr_ps = ps4.tile([1, 1], f32, tag="tp2")
            nc.tensor.matmul(swr_ps[:], wrs[:], ONESC[:])
            SWR = sb.tile([1, 1], f32)
            nc.vector.tensor_scalar(SWR[:], swr_ps[:], 2.0, None, Alu.mult)
            sb2_ps = ps4.tile([128, 1], f32, tag="tp2")
            nc.tensor.matmul(sb2_ps[:], ONES1[:], SWR[:])
            SCs2 = sb.tile([128, 1], f32)
            nc.scalar.copy(SCs2[:], sb2_ps[:])

            # parity m = S mod 2 (exact integer bit trick), chunked for overlap
            mag = sb.tile([128, P], f32)
            magu = sb.tile([128, P], u32)
            m_f = sb.tile([128, P], f32)
            for k in range(PT):
                ck = slice(k * 128, (k + 1) * 128)
                nc.vector.tensor_scalar(mag[:, ck], S_ps[:, ck], float(2 ** 23),
                                        None, Alu.add)
                nc.vector.tensor_scalar(magu[:, ck], mag[:, ck].bitcast(u32), 1,
                                        None, Alu.bitwise_and)
                nc.vector.tensor_copy(m_f[:, ck], magu[:, ck])
                mt_ps = ps2.tile([128, 128], f32, tag="tp")
                nc.tensor.transpose(mt_ps[:], m_f[:, ck], IDN)
                nc.scalar.copy(RHS[:, k, 0:128], mt_ps[:])
                ml_ps = ps4.tile([128, BS], f32, tag="tp2")
                nc.tensor.matmul(ml_ps[:], m_f[:, ck], EC)
                nc.scalar.copy(RHS[:, k, 128:NB], ml_ps[:])
            cnt = sb.tile([128, 1], f32)
            nc.vector.reduce_sum(cnt[:], m_f[:], axis=Ax.X)
            # per-row scale: alpha = 2*sum(Wr)*has_err (local rows; early)
            cl_ps = ps4.tile([BS, 1], f32, tag="tp2")
            nc.tensor.matmul(cl_ps[:], EC, cnt[:])
            HE = sb.tile([BS, 1], f32)
            nc.vector.tensor_scalar(HE[:], cl_ps[:], 0.0, None, Alu.is_gt)
            AL = sb.tile([BS, 1], f32)
            nc.vector.tensor_tensor(AL[:], HE[:], SCs2[0:BS, 0:1], Alu.mult)

            # ---- combined matmul over P (fp8 DoubleRow):  OUT = pc^T @ RHS ----
            WBA = sb.tile([128, LT, NW], f32)    # raw W-term columns + n
            WBCN = sb.tile([128, LT, 2], f32)    # reconstructed wb, wc per l
            BBT = sb.tile([128, LT, NB], f32)    # Bbias^T: full batch | local
            AMX = sb.tile([128, LT], f32)
            for t in range(LT):
                out_ps = ps.tile([128, NR], f32, tag="mm")
                for g in range(PT // 2):
                    nc.tensor.matmul(out_ps[:],
                                     PCL[:, 2 * g:2 * g + 2, t * 128:(t + 1) * 128],
                                     RHS[:, 2 * g:2 * g + 2, :],
                                     perf_mode=mybir.MatmulPerfMode.DoubleRow,
                                     start=(g == 0), stop=(g == PT // 2 - 1))
                nc.scalar.copy(WBA[:, t, :], out_ps[:, NB:NB + NW])
                # Bbias^T = n - 2*G^T on the scalar engine (keeps DVE free)
                nc.scalar.activation(BBT[:, t, :], out_ps[:, 0:NB], Act.Identity,
                                     bias=WBA[:, t, 8:9], scale=-2.0)
                nc.vector.tensor_reduce(AMX[:, t:t + 1], BBT[:, t, 0:128], axis=Ax.X,
                                        op=Alu.max, apply_absolute_value=True)
                if t % 2 == 1:
                    # wb,wc = ((t3/16 + t2)/16 + t1)/16 + t0, one tile-pair at a time
                    pr = slice(t - 1, t + 1)
                    nc.vector.scalar_tensor_tensor(WBCN[:, pr, :], WBA[:, pr, 6:8],
                                                   1.0 / 16.0, WBA[:, pr, 4:6],
                                                   Alu.mult, Alu.add)
                    nc.vector.scalar_tensor_tensor(WBCN[:, pr, :], WBCN[:, pr, :],
                                                   1.0 / 16.0, WBA[:, pr, 2:4],
                                                   Alu.mult, Alu.add)
                    nc.vector.scalar_tensor_tensor(WBCN[:, pr, :], WBCN[:, pr, :],
                                                   1.0 / 16.0, WBA[:, pr, 0:2],
                                                   Alu.mult, Alu.add)

            # ---- global 1/M, broadcast to partitions ----
            AMXr = sb.tile([128, 1], f32)
            nc.vector.tensor_reduce(AMXr[:], AMX[:], axis=Ax.X, op=Alu.max)
            tr_ps = ps4.tile([1, 128], f32, tag="tp2")
            nc.tensor.transpose(tr_ps[:], AMXr[:], IDN)
            Mg = sb.tile([1, 1], f32)
            nc.vector.tensor_reduce(Mg[:], tr_ps[:], axis=Ax.X, op=Alu.max)
            sb1_ps = ps4.tile([128, 1], f32, tag="tp2")
            nc.tensor.matmul(sb1_ps[:], ONES1[:], Mg[:])
            SCs1 = sb.tile([128, 1], f32)
            nc.vector.reciprocal(SCs1[:], sb1_ps[:])

            # ---- local elementwise (6+2 split: big part overlaps combined) ----
            XA = sb.tile([128, LT, BS], f32)
            A1 = sb.tile([128, LT, BS], f32)
            T1 = sb.tile([128, LT, BS], f32)
            A2 = sb.tile([128, LT, BS], f32)
            C2 = sb.tile([128, LT, BS], f32)
            U = sb.tile([128, LT, BS], f32)
            W = sb.tile([128, LT, BS], f32)
            Q8 = sb.tile([128, LT, BS], f32)
            qt_psA = ps3.tile([BS, 512], f32, tag="qta")
            qt_psB = ps3.tile([BS, 512], f32, tag="qtb")

            def flat(ap):
                return ap.rearrange("p i j -> p (i j)")

            # Everything except the 1/M scaling is M-independent:
            #   logits = alpha * (G1 + (1/M)*G2),  G1 = n*C*t1,  G2 = n*C*Bbias
            # G1 transposes start during the combined phase; the 1/M*G2 term
            # is folded in afterwards by accumulating G2 @ diag(1/M) onto the
            # same PSUM banks.
            CN = U   # reuse tiles
            G1 = W
            G2 = Q8
            for s0 in range(0, LT, 4):
                hs = slice(s0, s0 + 4)
                nc.scalar.activation(flat(XA[:, hs, :]), flat(XL[:, hs, :]), Act.Abs)
                nc.vector.tensor_tensor(A1[:, hs, :], XA[:, hs, :],
                                        bcast(WBCN[:, hs, 0:1], BS), Alu.mult)
                nc.scalar.activation(flat(T1[:, hs, :]), flat(A1[:, hs, :]), Act.Tanh)
                nc.vector.tensor_tensor(A2[:, hs, :], XA[:, hs, :],
                                        bcast(WBCN[:, hs, 1:2], BS), Alu.mult)
                nc.scalar.activation(flat(C2[:, hs, :]), flat(A2[:, hs, :]), Act.Tanh)
                nc.vector.scalar_tensor_tensor(CN[:, hs, :], C2[:, hs, :], 0.5,
                                               bcast(WBA[:, hs, 8:9], BS),
                                               Alu.add, Alu.mult)
                nc.vector.tensor_tensor(G1[:, hs, :], CN[:, hs, :], T1[:, hs, :],
                                        Alu.mult)
                nc.vector.tensor_tensor(G2[:, hs, :], CN[:, hs, :],
                                        BBT[:, hs, 128:NB], Alu.mult)
            # post-M: QS = G1 + (1/M)*G2, then transpose per tile
            QS = sb.tile([128, LT, BS], f32)
            nc.vector.scalar_tensor_tensor(QS[:], G2[:], SCs1[:, 0:1], G1[:],
                                           Alu.mult, Alu.add)
            for j in range(LT):
                qp, jo = (qt_psA, j) if j < 4 else (qt_psB, j - 4)
                nc.tensor.transpose(qp[:, jo * 128:(jo + 1) * 128], QS[:, j, :], IDN)

            # ---- alpha & softmax (halves split across ACT and DVE) ----
            QFa = sb.tile([BS, 512], f32)
            QFb = sb.tile([BS, 512], f32)
            nc.scalar.activation(QFa[:], qt_psA[:], Act.Copy, scale=AL[:, 0:1])
            nc.vector.tensor_scalar(QFb[:], qt_psB[:], AL[:, 0:1], None, Alu.mult)
            nma = sb.tile([BS, 1], f32)
            nmb = sb.tile([BS, 1], f32)
            nc.vector.tensor_reduce(nmb[:], QFb[:], axis=Ax.X, op=Alu.max, negate=True)
            nc.vector.tensor_reduce(nma[:], QFa[:], axis=Ax.X, op=Alu.max, negate=True)
            nmx = sb.tile([BS, 1], f32)
            nc.vector.tensor_tensor(nmx[:], nma[:], nmb[:], Alu.min)
            EXa = sb.tile([BS, 512], f32)
            EXb = sb.tile([BS, 512], f32)
            ssa = sb.tile([BS, 1], f32)
            ssb = sb.tile([BS, 1], f32)
            nc.scalar.activation(EXb[:], QFb[:], Act.Exp, bias=nmx[:, 0:1], scale=1.0)
            nc.vector.reduce_sum(ssb[:], EXb[:], axis=Ax.X)
            nc.scalar.activation(EXa[:], QFa[:], Act.Exp, bias=nmx[:, 0:1], scale=1.0)
            nc.vector.reduce_sum(ssa[:], EXa[:], axis=Ax.X)
            ssum = sb.tile([BS, 1], f32)
            nc.vector.tensor_tensor(ssum[:], ssa[:], ssb[:], Alu.add)
            rs = sb.tile([BS, 1], f32)
            nc.vector.reciprocal(rs[:], ssum[:])
            OUTa = sb.tile([BS, 512], f32)
            OUTb = sb.tile([BS, 512], f32)
            nc.vector.tensor_scalar(OUTb[:], EXb[:], rs[:, 0:1], None, Alu.mult)
            nc.scalar.activation(OUTa[:], EXa[:], Act.Copy, scale=rs[:, 0:1])
            nc.sync.dma_start(y_d[:, 0:512], OUTa[:])
            nc.sync.dma_start(y_d[:, 512:1024], OUTb[:])

    nc.compile()
    return nc


def _prep_in_maps(X, pc_matrix, Wb, Wc, Wr, br):
    bf16 = ml_dtypes.bfloat16
    fp8 = ml_dtypes.float8_e4m3
    X = np.ascontiguousarray(np.asarray(X, dtype=np.float32))
    pc = np.asarray(pc_matrix)
    xT = X[:, :, 0].T  # (L, B)

    xtb = np.ascontiguousarray(
        xT.astype(bf16).reshape(LT, 128, B).transpose(1, 0, 2).reshape(128, LT * B))
    pct = np.ascontiguousarray(
        pc.T.astype(fp8).reshape(LT, 128, P).transpose(1, 0, 2).reshape(128, LT * P))
    pcl = np.ascontiguousarray(
        pc.astype(fp8).reshape(PT, 128, L).transpose(1, 0, 2).reshape(128, PT * L))
    w3 = np.stack([np.asarray(Wb, dtype=np.float32)[0],
                   np.asarray(Wc, dtype=np.float32)[0]], axis=1)  # (P, 2)
    wt = w3.reshape(PT, 128, 2).transpose(1, 0, 2).reshape(128, PT * 2)
    wrp = np.asarray(Wr, dtype=np.float32).reshape(128, 4)
    idn = np.eye(128, dtype=np.float32)

    in_maps = []
    for c in range(NCORES):
        sel = slice(c * BS, (c + 1) * BS)
        ec = np.zeros((128, BS), dtype=np.float32)
        ec[np.arange(c * BS, (c + 1) * BS), np.arange(BS)] = 1.0
        xl = xT[:, sel].reshape(LT, 128, BS).transpose(1, 0, 2).reshape(128, LT * BS)
        big = np.concatenate([xl, ec, wt, wrp, idn], axis=1).astype(np.float32)
        assert big.shape == (128, 284)
        in_maps.append({"xtb": xtb, "pct": pct, "pcl": pcl,
                        "big": np.ascontiguousarray(big)})
    return in_maps


def run(inputs, trace=False, **kw):
    if "nc" not in _cache:
        _cache["nc"] = _build_nc()
    nc = _cache["nc"]
    in_maps = _prep_in_maps(**inputs)
    from concourse.bass_utils import run_bass_kernel_spmd
    res = run_bass_kernel_spmd(nc, in_maps, core_ids=list(range(NCORES)),
                               trace=trace, **kw)
    out = np.concatenate([res.results[c]["y"] for c in range(NCORES)], axis=0)
    return np.ascontiguousarray(out[:, :, None].astype(np.float32)), res


def kernel(**inputs) -> np.ndarray:
    out, _ = run(inputs)
    return out


# revision 23
# speedup vs baseline: 1.0513x; 1.0043x over previous
"""Trainium2 Bass kernel for nn_EncoderLayer_42399917146737.

The reference "SSM scan" is degenerate: at every step i the recurrence
overwrites h at exactly the positions p with pc[p,i]==1 with the scalar
b_i, and the step output reads only those positions.  Hence

    y_i[b] = C[b,i] * Bcoef[b,i] * n_i,      n_i = sum_p pc[p,i]

with no sequential dependence, and the reverse scan equals the forward
one.  The broadcast over p then reduces the Wr projection to a scalar
sum, so the whole module collapses to

    logits[b,l] = 2*sum(Wr) * has_err[b] * n_l * C[b,l] * (Bbias[b,l]/M + tanh(|X[b,l]|*wb_l))
    out         = softmax_l(logits)

where  Bbias = h0 @ pc,  h0 = 1-2*parity(hard @ pc^T),  hard = (X<0),
M = max|Bbias| (GLOBAL over the full batch),  wb = Wb @ pc,  wc = Wc @ pc,
C = 0.5 + tanh(|X|*wc_l).  (br shifts all logits equally -> drops out of
softmax.)

Sharding: batch B=128 over 8 cores (16 rows each).  Because M is a
global max over the whole batch, every core recomputes the (cheap)
full-batch parity/Bbias matmuls; the per-batch elementwise work + softmax
run only on the core's own 16 rows.  Per-core batch selection is done
with a per-core one-hot selection matrix (E_c) fed through the tensor
engine, so a single NEFF serves all 8 cores.

Precision: pc/hard/m are {0,1} so fp8/bf16 matmuls with f32 accumulate
are exact; X^T for sign tests rides in bf16 (sign-exact); Wb/Wc ride the
bf16 `pcl` matmul as hi+lo split columns (~2^-16 rel err); the local
elementwise path keeps full f32 X.
"""

import numpy as np
import ml_dtypes

B, L, P = 128, 1024, 512
NCORES = 8
BS = B // NCORES  # 16
LT = L // 128     # 8 L-tiles
PT = P // 128     # 4 P-tiles

_cache = {}


def _build_nc():
    import concourse.bass as bass
    import concourse.bacc as bacc
    import concourse.tile as tile
    from concourse import mybir

    f32 = mybir.dt.float32
    bf16 = mybir.dt.bfloat16
    fp8 = mybir.dt.float8e4
    u32 = mybir.dt.uint32
    Alu = mybir.AluOpType
    Act = mybir.ActivationFunctionType
    Ax = mybir.AxisListType

    nc = bacc.Bacc("TRN2", target_bir_lowering=False, debug=False)

    # ---- DRAM I/O (host pre-swizzles everything partition-major) ----
    xtb_d = nc.dram_tensor("xtb", (128, L), bf16, kind="ExternalInput")
    pct_d = nc.dram_tensor("pct", (128, LT * P), fp8, kind="ExternalInput")
    pcl_d = nc.dram_tensor("pcl", (128, PT * L), fp8, kind="ExternalInput")
    # bigf: [xl 0:128 | ec 128:144 | wt 144:152 | wr 152:156 | idn 156:284]
    NF = 284
    big_d = nc.dram_tensor("big", (128, NF), f32, kind="ExternalInput")
    y_d = nc.dram_tensor("y", (BS, L), f32, kind="ExternalOutput")

    NW = 9                    # wb0 wc0 wb1 wc1 wb2 wc2 wb3 wc3 | ones
    NB = 128 + BS             # m^T | m^T_loc
    NR = NB + NW              # combined-matmul rhs width
    HLT = LT // 2

    def bcast(col_ap, n):
        """Free-dim step-0 broadcast of a (...,1) AP to (...,n)."""
        return bass.AP(tensor=col_ap.tensor, offset=col_ap.offset,
                       ap=[*col_ap.ap[:-1], [0, n]])

    with tile.TileContext(nc) as tc:
        with (
            tc.tile_pool(name="sb", bufs=1) as sb,
            tc.tile_pool(name="ps", bufs=3, space="PSUM") as ps,
            tc.tile_pool(name="ps2", bufs=2, space="PSUM") as ps2,
            tc.tile_pool(name="ps4", bufs=1, space="PSUM") as ps4,
            tc.tile_pool(name="ps3", bufs=1, space="PSUM") as ps3,
        ):
            XTB = sb.tile([128, LT, 128], bf16)
            PCT = sb.tile([128, LT, P], fp8)
            PCL = sb.tile([128, PT, L], fp8)
            BIG = sb.tile([128, NF], f32)
            XL = BIG[:, 0:128].rearrange("p (i j) -> p i j", i=LT)
            EC = BIG[:, 128:144]
            WT = BIG[:, 144:152].rearrange("p (k t) -> p k t", k=PT)
            WRp = BIG[:, 152:156]
            IDN = BIG[:, 156:284]
            # One HWDGE ring; FIFO order = transfer priority.
            nc.sync.dma_start(XTB[:, 0:4, :].rearrange("p i b -> p (i b)"),
                              xtb_d[:, 0:512])
            nc.sync.dma_start(PCT[:, 0:4, :].rearrange("p i q -> p (i q)"),
                              pct_d[:, 0:4 * P])
            nc.sync.dma_start(XTB[:, 4:8, :].rearrange("p i b -> p (i b)"),
                              xtb_d[:, 512:1024])
            nc.sync.dma_start(PCT[:, 4:8, :].rearrange("p i q -> p (i q)"),
                              pct_d[:, 4 * P:8 * P])
            nc.sync.dma_start(BIG[:], big_d[:])
            nc.sync.dma_start(PCL[:].rearrange("p k l -> p (k l)"), pcl_d[:])

            # ---- hard decisions (transposed, fp8 {0,1}) ----
            HT = sb.tile([128, LT, 128], fp8)
            for h in range(2):
                nc.vector.tensor_scalar(
                    HT[:, 4 * h:4 * h + 4, :].rearrange("p i b -> p (i b)"),
                    XTB[:, 4 * h:4 * h + 4, :].rearrange("p i b -> p (i b)"),
                    0.0, None, Alu.is_lt)

            # ---- syndrome counts: S[b,q] = sum_l hard[b,l]*pc[q,l] ----
            S_ps = ps.tile([128, P], f32, tag="mm")
            for g in range(LT // 2):
                nc.tensor.matmul(S_ps[:], HT[:, 2 * g:2 * g + 2, :],
                                 PCT[:, 2 * g:2 * g + 2, :],
                                 perf_mode=mybir.MatmulPerfMode.DoubleRow,
                                 start=(g == 0), stop=(g == LT // 2 - 1))

            # ---- combined rhs (fp8): [ m^T | m^T_loc | W 4-term splits | ones ]
            # Wb/Wc are carried as 4 scaled fp8 terms each: w = sum_k t_k/16^k,
            # with t_k stored as fp8(residual_k * 16^k) so terms stay in
            # fp8's normal range.  Reconstruction happens after the matmul.
            RHS = sb.tile([128, PT, NR], fp8)
            R1 = sb.tile([128, PT, 2], f32)
            R2 = sb.tile([128, PT, 2], f32)
            R3 = sb.tile([128, PT, 2], f32)
            for k in range(PT):
                nc.scalar.copy(RHS[:, k, NB:NB + 2], WT[:, k, :])            # t0
                nc.vector.tensor_tensor(R1[:, k, :], WT[:, k, :],
                                        RHS[:, k, NB:NB + 2], Alu.subtract)
                nc.vector.tensor_scalar(RHS[:, k, NB + 2:NB + 4], R1[:, k, :],
                                        16.0, None, Alu.mult)                # t1
                nc.vector.scalar_tensor_tensor(R2[:, k, :],
                                               RHS[:, k, NB + 2:NB + 4],
                                               -1.0 / 16.0, R1[:, k, :],
                                               Alu.mult, Alu.add)
                nc.vector.tensor_scalar(RHS[:, k, NB + 4:NB + 6], R2[:, k, :],
                                        256.0, None, Alu.mult)               # t2
                nc.vector.scalar_tensor_tensor(R3[:, k, :],
                                               RHS[:, k, NB + 4:NB + 6],
                                               -1.0 / 256.0, R2[:, k, :],
                                               Alu.mult, Alu.add)
                nc.vector.tensor_scalar(RHS[:, k, NB + 6:NB + 8], R3[:, k, :],
                                        4096.0, None, Alu.mult)              # t3
                nc.vector.memset(RHS[:, k, NB + 8:NB + 9], 1.0)              # ones
            # early scalar chain: 2*sum(Wr) broadcast (independent of parity/M)
            ONES1 = sb.tile([1, 128], f32)
            nc.vector.memset(ONES1[:], 1.0)
            ONESC = sb.tile([128, 1], f32)
            nc.vector.memset(ONESC[:], 1.0)
            wrs = sb.tile([128, 1], f32)
            nc.vector.reduce_sum(wrs[:], WRp, axis=Ax.X)
            swr_ps = ps4.tile([1, 1], f32, tag="tp2")
            nc.tensor.matmul(swr_ps[:], wrs[:], ONESC[:])
            SWR = sb.tile([1, 1], f32)
            nc.vector.tensor_scalar(SWR[:], swr_ps[:], 2.0, None, Alu.mult)
            sb2_ps = ps4.tile([128, 1], f32, tag="tp2")
            nc.tensor.matmul(sb2_ps[:], ONES1[:], SWR[:])
            SCs2 = sb.tile([128, 1], f32)
            nc.scalar.copy(SCs2[:], sb2_ps[:])

            # parity m = S mod 2 (exact integer bit trick), chunked for overlap
            mag = sb.tile([128, P], f32)
            magu = sb.tile([128, P], u32)
            m_f = sb.tile([128, P], f32)
            for k in range(PT):
                ck = slice(k * 128, (k + 1) * 128)
                nc.vector.tensor_scalar(mag[:, ck], S_ps[:, ck], float(2 ** 23),
                                        None, Alu.add)
                nc.vector.tensor_scalar(magu[:, ck], mag[:, ck].bitcast(u32), 1,
                                        None, Alu.bitwise_and)
                nc.vector.tensor_copy(m_f[:, ck], magu[:, ck])
                mt_ps = ps2.tile([128, 128], f32, tag="tp")
                nc.tensor.transpose(mt_ps[:], m_f[:, ck], IDN)
                nc.scalar.copy(RHS[:, k, 0:128], mt_ps[:])
                ml_ps = ps4.tile([128, BS], f32, tag="tp2")
                nc.tensor.matmul(ml_ps[:], m_f[:, ck], EC)
                nc.scalar.copy(RHS[:, k, 128:NB], ml_ps[:])
            cnt = sb.tile([128, 1], f32)
            nc.vector.reduce_sum(cnt[:], m_f[:], axis=Ax.X)
            # per-row scale: alpha = 2*sum(Wr)*has_err (local rows; early)
            cl_ps = ps4.tile([BS, 1], f32, tag="tp2")
            nc.tensor.matmul(cl_ps[:], EC, cnt[:])
            HE = sb.tile([BS, 1], f32)
            nc.vector.tensor_scalar(HE[:], cl_ps[:], 0.0, None, Alu.is_gt)
            AL = sb.tile([BS, 1], f32)
            nc.vector.tensor_tensor(AL[:], HE[:], SCs2[0:BS, 0:1], Alu.mult)

            # ---- combined matmul over P (fp8 DoubleRow):  OUT = pc^T @ RHS ----
            WBA = sb.tile([128, LT, NW], f32)    # raw W-term columns + n
            WBCN = sb.tile([128, LT, 2], f32)    # reconstructed wb, wc per l
            BBT = sb.tile([128, LT, NB], f32)    # Bbias^T: full batch | local
            AMX = sb.tile([128, LT], f32)
            for t in range(LT):
                out_ps = ps.tile([128, NR], f32, tag="mm")
                for g in range(PT // 2):
                    nc.tensor.matmul(out_ps[:],
                                     PCL[:, 2 * g:2 * g + 2, t * 128:(t + 1) * 128],
                                     RHS[:, 2 * g:2 * g + 2, :],
                                     perf_mode=mybir.MatmulPerfMode.DoubleRow,
                                     start=(g == 0), stop=(g == PT // 2 - 1))
                nc.scalar.copy(WBA[:, t, :], out_ps[:, NB:NB + NW])
                # Bbias^T = n - 2*G^T on the scalar engine (keeps DVE free)
                nc.scalar.activation(BBT[:, t, :], out_ps[:, 0:NB], Act.Identity,
                                     bias=WBA[:, t, 8:9], scale=-2.0)
                nc.vector.tensor_reduce(AMX[:, t:t + 1], BBT[:, t, 0:128], axis=Ax.X,
                                        op=Alu.max, apply_absolute_value=True)
                if t % 2 == 1:
                    # wb,wc = ((t3/16 + t2)/16 + t1)/16 + t0, one tile-pair at a time
                    pr = slice(t - 1, t + 1)
                    nc.vector.scalar_tensor_tensor(WBCN[:, pr, :], WBA[:, pr, 6:8],
                                                   1.0 / 16.0, WBA[:, pr, 4:6],
                                                   Alu.mult, Alu.add)
                    nc.vector.scalar_tensor_tensor(WBCN[:, pr, :], WBCN[:, pr, :],
                                                   1.0 / 16.0, WBA[:, pr, 2:4],
                                                   Alu.mult, Alu.add)
                    nc.vector.scalar_tensor_tensor(WBCN[:, pr, :], WBCN[:, pr, :],
                                                   1.0 / 16.0, WBA[:, pr, 0:2],
                                                   Alu.mult, Alu.add)

            # ---- global 1/M, broadcast to partitions ----
            AMXr = sb.tile([128, 1], f32)
            nc.vector.tensor_reduce(AMXr[:], AMX[:], axis=Ax.X, op=Alu.max)
            tr_ps = ps4.tile([1, 128], f32, tag="tp2")
            nc.tensor.transpose(tr_ps[:], AMXr[:], IDN)
            Mg = sb.tile([1, 1], f32)
            nc.vector.tensor_reduce(Mg[:], tr_ps[:], axis=Ax.X, op=Alu.max)
            sb1_ps = ps4.tile([128, 1], f32, tag="tp2")
            nc.tensor.matmul(sb1_ps[:], ONES1[:], Mg[:])
            SCs1 = sb.tile([128, 1], f32)
            nc.vector.reciprocal(SCs1[:], sb1_ps[:])

            # ---- local elementwise (6+2 split: big part overlaps combined) ----
            XA = sb.tile([128, LT, BS], f32)
            A1 = sb.tile([128, LT, BS], f32)
            T1 = sb.tile([128, LT, BS], f32)
            A2 = sb.tile([128, LT, BS], f32)
            C2 = sb.tile([128, LT, BS], f32)
            U = sb.tile([128, LT, BS], f32)
            W = sb.tile([128, LT, BS], f32)
            Q8 = sb.tile([128, LT, BS], f32)
            qt_psA = ps3.tile([BS, 512], f32, tag="qta")
            qt_psB = ps3.tile([BS, 512], f32, tag="qtb")

            def flat(ap):
                return ap.rearrange("p i j -> p (i j)")

            # QS = n*C*(t1 + Bbias/M) = CN*(T1 + invM*BBL); only the last
            # two ops are gated on M.
            CN = U   # reuse tiles
            for s0 in range(0, LT, 4):
                hs = slice(s0, s0 + 4)
                nc.scalar.activation(flat(XA[:, hs, :]), flat(XL[:, hs, :]), Act.Abs)
                nc.vector.tensor_tensor(A1[:, hs, :], XA[:, hs, :],
                                        bcast(WBCN[:, hs, 0:1], BS), Alu.mult)
                nc.scalar.activation(flat(T1[:, hs, :]), flat(A1[:, hs, :]), Act.Tanh)
                nc.vector.tensor_tensor(A2[:, hs, :], XA[:, hs, :],
                                        bcast(WBCN[:, hs, 1:2], BS), Alu.mult)
                nc.scalar.activation(flat(C2[:, hs, :]), flat(A2[:, hs, :]), Act.Tanh)
                nc.vector.scalar_tensor_tensor(CN[:, hs, :], C2[:, hs, :], 0.5,
                                               bcast(WBA[:, hs, 8:9], BS),
                                               Alu.add, Alu.mult)
            UU = W
            QS = Q8
            nc.vector.scalar_tensor_tensor(UU[:], BBT[:, :, 128:NB], SCs1[:, 0:1],
                                           T1[:], Alu.mult, Alu.add)
            nc.vector.tensor_tensor(QS[:], CN[:], UU[:], Alu.mult)
            for j in range(LT):
                qp, jo = (qt_psA, j) if j < 4 else (qt_psB, j - 4)
                nc.tensor.transpose(qp[:, jo * 128:(jo + 1) * 128], QS[:, j, :], IDN)

            # ---- alpha & softmax: quarters pipelined across ACT/DVE ----
            QFa = sb.tile([BS, 512], f32)
            QFb = sb.tile([BS, 512], f32)
            nm4 = sb.tile([BS, 4], f32)
            for q in range(2):
                cq = slice(q * 256, (q + 1) * 256)
                nc.scalar.activation(QFa[:, cq], qt_psA[:, cq], Act.Copy,
                                     scale=AL[:, 0:1])
                nc.vector.tensor_reduce(nm4[:, q:q + 1], QFa[:, cq], axis=Ax.X,
                                        op=Alu.max, negate=True)
            for q in range(2):
                cq = slice(q * 256, (q + 1) * 256)
                nc.vector.tensor_scalar(QFb[:, cq], qt_psB[:, cq], AL[:, 0:1],
                                        None, Alu.mult)
                nc.vector.tensor_reduce(nm4[:, 2 + q:3 + q], QFb[:, cq], axis=Ax.X,
                                        op=Alu.max, negate=True)
            nmx = sb.tile([BS, 1], f32)
            nc.vector.tensor_reduce(nmx[:], nm4[:], axis=Ax.X, op=Alu.min)
            EXa = sb.tile([BS, 512], f32)
            EXb = sb.tile([BS, 512], f32)
            ssa = sb.tile([BS, 1], f32)
            ssb = sb.tile([BS, 1], f32)
            nc.scalar.activation(EXb[:], QFb[:], Act.Exp, bias=nmx[:, 0:1], scale=1.0,
                                 accum_out=ssb[:])
            nc.scalar.activation(EXa[:], QFa[:], Act.Exp, bias=nmx[:, 0:1], scale=1.0,
                                 accum_out=ssa[:])
            ssum = sb.tile([BS, 1], f32)
            nc.vector.tensor_tensor(ssum[:], ssa[:], ssb[:], Alu.add)
            rs = sb.tile([BS, 1], f32)
            nc.vector.reciprocal(rs[:], ssum[:])
            OUTa = sb.tile([BS, 512], f32)
            OUTb = sb.tile([BS, 512], f32)
            nc.vector.tensor_scalar(OUTb[:], EXb[:], rs[:, 0:1], None, Alu.mult)
            nc.scalar.activation(OUTa[:], EXa[:], Act.Copy, scale=rs[:, 0:1])
            nc.sync.dma_start(y_d[:, 0:512], OUTa[:])
            nc.sync.dma_start(y_d[:, 512:1024], OUTb[:])

    nc.compile()
    return nc


def _prep_in_maps(X, pc_matrix, Wb, Wc, Wr, br):
    bf16 = ml_dtypes.bfloat16
    fp8 = ml_dtypes.float8_e4m3
    X = np.ascontiguousarray(np.asarray(X, dtype=np.float32))
    pc = np.asarray(pc_matrix)
    xT = X[:, :, 0].T  # (L, B)

    xtb = np.ascontiguousarray(
        xT.astype(bf16).reshape(LT, 128, B).transpose(1, 0, 2).reshape(128, LT * B))
    pct = np.ascontiguousarray(
        pc.T.astype(fp8).reshape(LT, 128, P).transpose(1, 0, 2).reshape(128, LT * P))
    pcl = np.ascontiguousarray(
        pc.astype(fp8).reshape(PT, 128, L).transpose(1, 0, 2).reshape(128, PT * L))
    w3 = np.stack([np.asarray(Wb, dtype=np.float32)[0],
                   np.asarray(Wc, dtype=np.float32)[0]], axis=1)  # (P, 2)
    wt = w3.reshape(PT, 128, 2).transpose(1, 0, 2).reshape(128, PT * 2)
    wrp = np.asarray(Wr, dtype=np.float32).reshape(128, 4)
    idn = np.eye(128, dtype=np.float32)

    in_maps = []
    for c in range(NCORES):
        sel = slice(c * BS, (c + 1) * BS)
        ec = np.zeros((128, BS), dtype=np.float32)
        ec[np.arange(c * BS, (c + 1) * BS), np.arange(BS)] = 1.0
        xl = xT[:, sel].reshape(LT, 128, BS).transpose(1, 0, 2).reshape(128, LT * BS)
        big = np.concatenate([xl, ec, wt, wrp, idn], axis=1).astype(np.float32)
        assert big.shape == (128, 284)
        in_maps.append({"xtb": xtb, "pct": pct, "pcl": pcl,
                        "big": np.ascontiguousarray(big)})
    return in_maps


def run(inputs, trace=False, **kw):
    if "nc" not in _cache:
        _cache["nc"] = _build_nc()
    nc = _cache["nc"]
    in_maps = _prep_in_maps(**inputs)
    from concourse.bass_utils import run_bass_kernel_spmd
    res = run_bass_kernel_spmd(nc, in_maps, core_ids=list(range(NCORES)),
                               trace=trace, **kw)
    out = np.concatenate([res.results[c]["y"] for c in range(NCORES)], axis=0)
    return np.ascontiguousarray(out[:, :, None].astype(np.float32)), res


def kernel(**inputs) -> np.ndarray:
    out, _ = run(inputs)
    return out
